# revision 17
# baseline (speedup 1.0000x reference)
"""MiniMax-M2 sparse MoE block on 8 Trainium2 NeuronCores.

Expert-parallel, sparse (routed) compute, host-side dispatch:
  - Host computes the router (sigmoid + top-2) in f64, gathers each
    expert's assigned tokens (padded to capacity C=96; max observed
    count is 75), dequantizes the fp8-block-quant weights (w * scale),
    casts everything to bf16, and lays out all tensors in the exact
    SBUF-tile layouts the device consumes (all transposes happen on
    host during sharding — pure layout work, no math beyond dequant).
  - Each core owns 2 of 16 experts and streams ~20 MB of bf16 from HBM
    per invocation:
      xet [128h, e, hb, c]          gathered tokens, transposed
      w1t/w3t [e, 128h, it, hb, ci] dequantized W1/W3, transposed,
                                    it-major so DMA chunks split on it
      w2t [e, 128i, ht, ib, ch]     dequantized W2, transposed, ht-major
    Weight DMA chunks rotate across the three descriptor paths
    (SP HWDGE, ACT HWDGE, SWDGE) so transfers run concurrently.
    Per expert: gT/uT [128i, c] psums accumulate over 16 h-blocks
    (weight tile stationary, tokens moving); a = silu(g) * u in bf16;
    down proj yT [128h, c] accumulates over 6 i-blocks; y stored bf16.
  - Host scatter-adds expert outputs back with the combine weights
    (top-2 sum-normalized sigmoid scores) during unsharding.
  - Overflow seatbelt: if an expert is assigned more than C tokens the
    remainder are computed exactly on host in numpy and added in.
"""

import os
import sys
import numpy as np

for _p in ("/opt/trn_rl_repo", "/root/.axon_site/_ro/trn_rl_repo"):
    if os.path.isdir(_p) and _p not in sys.path:
        sys.path.insert(0, _p)
        break

import ml_dtypes

T, H, I, E = 512, 2048, 768, 16
NCORES, EPC = 8, 2
P = 128
HB, IB = H // P, I // P          # 16, 6
C = 96                           # per-expert token capacity (max seen: 75)
ITCH = IB // 2                   # it-chunk for w1/w3 DMA halves
HTCH = HB // 2                   # ht-chunk for w2 DMA halves
BLOCK = 128                      # fp8 block-quant block size

_CACHE = {}


def _emit_body(nc, mybir, pools, dram):
    f32 = mybir.dt.float32
    bf16 = mybir.dt.bfloat16
    AF = mybir.ActivationFunctionType
    OP = mybir.AluOpType
    (xp, wp, w2p, ap, sp, yp, ps) = pools
    (xet_d, w1_d, w3_d, w2_d, y_d) = dram

    # rotate weight-chunk DMAs across the three descriptor paths
    # (SP HWDGE ring, ACT HWDGE ring, SWDGE) so transfers overlap
    qs = [nc.sync, nc.scalar, nc.gpsimd]
    qi = [0]

    def wdma(tile_, src):
        qs[qi[0] % 3].dma_start(tile_, src)
        qi[0] += 1

    xet = xp.tile([P, EPC, HB, C], bf16, tag="xet", name="xet")
    nc.gpsimd.dma_start(xet[:], xet_d[:])

    for e in range(EPC):
        # chunks split along the psum dim (it/ht) so the first matmul
        # group only depends on the first chunk
        w1c, w3c = [], []
        for half in range(2):
            t1 = wp.tile([P, ITCH, HB, P], bf16, tag="w13", name="w1c")
            wdma(t1[:], w1_d[e, :, half * ITCH:(half + 1) * ITCH])
            w1c.append(t1)
            t3 = wp.tile([P, ITCH, HB, P], bf16, tag="w13", name="w3c")
            wdma(t3[:], w3_d[e, :, half * ITCH:(half + 1) * ITCH])
            w3c.append(t3)
        w2c = []
        for half in range(2):
            t2 = w2p.tile([P, HTCH, IB, P], bf16, tag="w2", name="w2c")
            wdma(t2[:], w2_d[e, :, half * HTCH:(half + 1) * HTCH])
            w2c.append(t2)

        # up/gate projections: gT/uT [128 i, C] accumulated over 16 h-blocks
        aT = []
        for it in range(IB):
            pg = ps.tile([P, C], f32, tag="ps", name="pg")
            pu = ps.tile([P, C], f32, tag="ps", name="pu")
            for hb in range(HB):
                nc.tensor.matmul(pg[:], w1c[it // ITCH][:, it % ITCH, hb, :],
                                 xet[:, e, hb, :],
                                 start=(hb == 0), stop=(hb == HB - 1))
            for hb in range(HB):
                nc.tensor.matmul(pu[:], w3c[it // ITCH][:, it % ITCH, hb, :],
                                 xet[:, e, hb, :],
                                 start=(hb == 0), stop=(hb == HB - 1))
            sg = sp.tile([P, C], bf16, tag="sg", name="sg")
            nc.scalar.activation(sg[:], pg[:], AF.Sigmoid)
            xs = sp.tile([P, C], bf16, tag="xs", name="xs")
            nc.vector.tensor_tensor(out=xs[:], in0=sg[:], in1=pg[:],
                                    op=OP.mult)
            a = ap.tile([P, C], bf16, tag="aT", name="aT")
            nc.vector.tensor_tensor(out=a[:], in0=xs[:], in1=pu[:],
                                    op=OP.mult)
            aT.append(a)

        # down projection: yT [128 h, C] accumulated over 6 i-blocks
        ys = yp.tile([P, HB, C], bf16, tag="ys", name="ys")
        for ht in range(HB):
            py = ps.tile([P, C], f32, tag="ps", name="py")
            for ib in range(IB):
                nc.tensor.matmul(py[:], w2c[ht // HTCH][:, ht % HTCH, ib, :],
                                 aT[ib][:],
                                 start=(ib == 0), stop=(ib == IB - 1))
            nc.vector.tensor_copy(ys[:, ht, :], py[:])
        eng = nc.sync if e == 0 else nc.scalar
        eng.dma_start(y_d[e], ys[:])


def build_nc(reps=1):
    import concourse.bacc as bacc
    import concourse.mybir as mybir
    import concourse.tile as tile
    from contextlib import ExitStack

    bf16 = mybir.dt.bfloat16

    nc = bacc.Bacc("TRN2", target_bir_lowering=False, debug=False,
                   num_devices=NCORES)

    xet_d = nc.dram_tensor("xet", [P, EPC, HB, C], bf16, kind="ExternalInput")
    w1_d = nc.dram_tensor("w1t", [EPC, P, IB, HB, P], bf16,
                          kind="ExternalInput")
    w3_d = nc.dram_tensor("w3t", [EPC, P, IB, HB, P], bf16,
                          kind="ExternalInput")
    w2_d = nc.dram_tensor("w2t", [EPC, P, HB, IB, P], bf16,
                          kind="ExternalInput")
    y_d = nc.dram_tensor("y", [EPC, P, HB, C], bf16, kind="ExternalOutput")
    dram = (xet_d, w1_d, w3_d, w2_d, y_d)

    with tile.TileContext(nc) as tc:
        with ExitStack() as ctx:
            pools = (
                ctx.enter_context(tc.tile_pool(name="xet", bufs=2)),
                ctx.enter_context(tc.tile_pool(name="w13", bufs=6)),
                ctx.enter_context(tc.tile_pool(name="w2", bufs=4)),
                ctx.enter_context(tc.tile_pool(name="aT", bufs=IB + 2)),
                ctx.enter_context(tc.tile_pool(name="sact", bufs=3)),
                ctx.enter_context(tc.tile_pool(name="ys", bufs=2)),
                ctx.enter_context(tc.tile_pool(name="ps", bufs=8,
                                               space="PSUM")),
            )
            for _rep in range(reps):
                _emit_body(nc, mybir, pools, dram)

    nc.compile()
    return nc


def _route(hidden_states, gate_w):
    """Top-2 sigmoid routing in f64. Returns (idx [E,C], counts [E],
    cw [E,C]) with idx padded with 0 and cw zero-padded; overflow tokens
    (beyond capacity) are returned separately as (eg, token, weight)."""
    x = np.asarray(hidden_states, np.float64).reshape(T, H)
    gw = np.asarray(gate_w, np.float64)
    s = 1.0 / (1.0 + np.exp(-(x @ gw.T)))            # [T, E]
    order = np.argsort(s, axis=-1)
    top2 = order[:, -2:]                             # [:, 1] is argmax
    tw = np.take_along_axis(s, top2, axis=-1)
    den = tw.sum(-1, keepdims=True)
    cwt = tw / den                                   # [T, 2]
    idx = np.zeros((E, C), np.int64)
    cw = np.zeros((E, C), np.float64)
    counts = np.zeros(E, np.int64)
    overflow = []
    for t in range(T):
        for k in range(2):
            e = int(top2[t, k])
            n = counts[e]
            if n < C:
                idx[e, n] = t
                cw[e, n] = cwt[t, k]
                counts[e] = n + 1
            else:
                overflow.append((e, t, cwt[t, k]))
    return idx, counts, cw, overflow


def _dequant(w, s):
    E_, O_, I_ = w.shape
    nb = s.shape[-1]
    bs = I_ // nb
    return (w.reshape(E_, O_, nb, bs) * s[..., None]).reshape(E_, O_, I_)


def shard_inputs(hidden_states, gate_w, w1, w1_scale, w3, w3_scale,
                 w2, w2_scale):
    bf = ml_dtypes.bfloat16
    x = np.asarray(hidden_states, np.float32).reshape(T, H)
    idx, counts, cw, overflow = _route(hidden_states, gate_w)

    w1d = _dequant(np.asarray(w1, np.float32), np.asarray(w1_scale, np.float32))
    w3d = _dequant(np.asarray(w3, np.float32), np.asarray(w3_scale, np.float32))
    w2d = _dequant(np.asarray(w2, np.float32), np.asarray(w2_scale, np.float32))

    in_maps = []
    for c in range(NCORES):
        lo = c * EPC
        # xet[p, el, hb, c] = x[idx[lo+el, c], hb*128+p]
        xg = x[idx[lo:lo + EPC].reshape(-1)]              # [EPC*C, H]
        xet = (xg.reshape(EPC, C, HB, P).transpose(3, 0, 2, 1))
        # w13t[el, p, it, hb, ci] = w_d[e, it*128+ci, hb*128+p]
        w1t = w1d[lo:lo + EPC].reshape(EPC, IB, P, HB, P).transpose(
            0, 4, 1, 3, 2)
        w3t = w3d[lo:lo + EPC].reshape(EPC, IB, P, HB, P).transpose(
            0, 4, 1, 3, 2)
        # w2t[el, p, ht, ib, ch] = w2_d[e, ht*128+ch, ib*128+p]
        w2t = w2d[lo:lo + EPC].reshape(EPC, HB, P, IB, P).transpose(
            0, 4, 1, 3, 2)
        in_maps.append({
            "xet": np.ascontiguousarray(xet).astype(bf),
            "w1t": np.ascontiguousarray(w1t).astype(bf),
            "w3t": np.ascontiguousarray(w3t).astype(bf),
            "w2t": np.ascontiguousarray(w2t).astype(bf),
        })
    return in_maps


def _host_expert(x_rows, w1d_e, w3d_e, w2d_e):
    """Exact single-expert MLP on host for overflow tokens."""
    g = x_rows @ w1d_e.T
    u = x_rows @ w3d_e.T
    a = (g / (1.0 + np.exp(-g))) * u
    return a @ w2d_e.T


def kernel(hidden_states, gate_w, w1, w1_scale, w3, w3_scale, w2, w2_scale,
           top_k):
    assert int(top_k) == 2
    from concourse.bass_utils import run_bass_kernel_spmd

    hidden_states = np.asarray(hidden_states)
    B, S, _ = hidden_states.shape
    if "nc" not in _CACHE:
        _CACHE["nc"] = build_nc()
    nc = _CACHE["nc"]

    idx, counts, cw, overflow = _route(hidden_states, gate_w)
    in_maps = shard_inputs(hidden_states, gate_w, w1, w1_scale,
                           w3, w3_scale, w2, w2_scale)
    res = run_bass_kernel_spmd(nc, in_maps, list(range(NCORES)))

    y = np.zeros((T, H), np.float64)
    for c in range(NCORES):
        r = np.asarray(res.results[c]["y"], dtype=np.float32)  # [EPC,P,HB,C]
        for el in range(EPC):
            eg = c * EPC + el
            n = int(min(counts[eg], C))
            if n == 0:
                continue
            ye = r[el].transpose(1, 0, 2).reshape(H, C)        # [H, C]
            y[idx[eg, :n]] += (ye[:, :n] * cw[eg, :n]).T

    if overflow:
        x64 = np.asarray(hidden_states, np.float64).reshape(T, H)
        w1d = _dequant(np.asarray(w1, np.float64),
                       np.asarray(w1_scale, np.float64))
        w3d = _dequant(np.asarray(w3, np.float64),
                       np.asarray(w3_scale, np.float64))
        w2d = _dequant(np.asarray(w2, np.float64),
                       np.asarray(w2_scale, np.float64))
        for eg, t, wgt in overflow:
            y[t] += wgt * _host_expert(x64[t:t + 1], w1d[eg], w3d[eg],
                                       w2d[eg])[0]

    return y.reshape(B, S, H).astype(np.float32)
